# revision 17
# baseline (speedup 1.0000x reference)
"""Trainium2 Bass kernel for single-head attention (nn_Attention_31344671326347).

Problem: B=4, S=2048, E=D=1024, fp32.
    q = x @ Wq.T + bq ; k = x @ Wk.T + bk ; v = x @ Wv.T + bv
    out = softmax(q k^T / sqrt(D)) @ v

Sharding: 8 cores = (4 batches) x (2 query-halves). v2: each core projects
K/V only for its OWN sequence half; the halves are exchanged between the
pair (2b, 2b+1) with pairwise AllGathers through DRAM bounce buffers, in
ascending-rank (= global) order, so the program stays SPMD-uniform: every
core reads back the full K/V in global order exactly like the baseline did.

All on-device tensors are bf16 (inputs converted on host); PSUM accumulation
stays fp32. This halves DMA/collective bytes and lets the full V stay
SBUF-resident after the gather. fp32 is kept for biases, PSUM, and the
final reciprocal/scale + output.

Layout trick (unchanged): contractions run with the contracted dim on SBUF
partitions; host ships x^T and W^T so q^T [d,s], k^T [d,t] and v [t,d] come
out of the PE with no on-device transposes; softmax runs over the partition
dim via exp (ScalarE) + ones-columns of V carrying the denominator through
the PV matmul.
"""

import numpy as np
import ml_dtypes

import concourse.bass as bass
import concourse.mybir as mybir
import concourse.tile as tile
from concourse import bacc
from concourse.bass_utils import run_bass_kernel_spmd

B, S, E, D = 4, 2048, 1024, 1024
SQ = S // 2          # query rows / own K,V rows per core
P = 128
EO = E // P          # 8 contraction chunks
DO = D // P          # 8 d chunks
TC = S // P          # 16 key/t chunks (global)
TB = 2               # 512-wide t blocks in own half
SB = SQ // 512       # 2 big s chunks
DA = 1032            # d + 8 ones columns (denominator rides the PV matmul)
DC = 344             # PV d-chunk width (3 * 344 = 1032)
BF = mybir.dt.bfloat16
F32 = mybir.dt.float32

N_CORES = 8
GROUPS = [[0, 1], [2, 3], [4, 5], [6, 7]]
TRACE = False        # test.py flips this for profiling
LAST_RESULT = None   # BassKernelResults of the most recent run

_NC = None


def _build():
    nc = bacc.Bacc("TRN2", target_bir_lowering=False, debug=False,
                   num_devices=N_CORES)

    xT = nc.dram_tensor("xT", [E, SQ], BF, kind="ExternalInput")
    wqT = nc.dram_tensor("wqT", [E, D], BF, kind="ExternalInput")
    wkT = nc.dram_tensor("wkT", [E, D], BF, kind="ExternalInput")
    wvT = nc.dram_tensor("wvT", [E, DA], BF, kind="ExternalInput")
    bq = nc.dram_tensor("bq", [P, DO], F32, kind="ExternalInput")
    bk = nc.dram_tensor("bk", [P, DO], F32, kind="ExternalInput")
    bv = nc.dram_tensor("bv", [P, DA], F32, kind="ExternalInput")
    out = nc.dram_tensor("out", [3, SQ, DC], F32, kind="ExternalOutput")

    xT_r = xT.rearrange("(eo p) s -> p eo s", p=P)
    w_r = {
        "q": wqT.rearrange("(eo p) d -> p eo d", p=P),
        "k": wkT.rearrange("(eo p) d -> p eo d", p=P),
        "v": wvT.rearrange("(eo p) d -> p eo d", p=P),
    }

    Ident = mybir.ActivationFunctionType.Identity
    Exp = mybir.ActivationFunctionType.Exp
    inv_sqrt_d = float(1.0 / np.sqrt(D))

    with tile.TileContext(nc) as tc:
        with (
            tc.tile_pool(name="res", bufs=1) as res,
            tc.tile_pool(name="small", bufs=1) as small,
            tc.tile_pool(name="dram", bufs=1, space="DRAM") as dram_pool,
        ):
            qT_t = res.tile([P, DO, SQ], BF, tag="qT")
            kT_t = res.tile([P, DO, S], BF, tag="kT")
            vF_t = res.tile([P, TC, DA], BF, tag="vF")

            bqk = small.tile([P, 2 * DO], F32, tag="bqk")
            bq_t = bqk[:, :DO]
            bk_t = bqk[:, DO:]
            bv_t = small.tile([P, DA], F32, tag="bv")

            # DRAM bounce buffers for the pairwise exchange; both K and V
            # are exchanged in two ~1MB AllGathers to pipeline on the CC
            # queue and start as early as possible. Each AllGather gets its
            # own DRAM tile: dep tracking on DRAM tiles is whole-tile, so a
            # shared tile would delay the first AG until ALL halves are
            # written.
            k_bnc = [dram_pool.tile([P, DO, 512], BF, name=f"k_bnc{i}")
                     for i in range(TB)]
            ag_k = [dram_pool.tile([2, P, DO, 512], BF, name=f"ag_k{i}")
                    for i in range(TB)]
            v_bnc = [dram_pool.tile([512, DA], BF, name=f"v_bnc{i}")
                     for i in range(TB)]
            ag_v = [dram_pool.tile([2, 512, DA], BF, name=f"ag_v{i}")
                    for i in range(TB)]
            v_bnc_r = [t.rearrange("(tc p) d -> p tc d", p=P) for t in v_bnc]
            ag_v_r = [t.rearrange("g (tc p) d -> g p tc d", p=P)
                      for t in ag_v]

            # ---- projections (K-own, V-own, Q) ----
            # K runs first: its AllGathers claim the CC queue as soon as the
            # kernel-entry collective barrier (~45us) clears, since QK needs
            # kT earliest; V's AGs follow; Q needs no exchange so it is last.
            with (
                tc.tile_pool(name="wpool", bufs=3) as wpool,
                tc.tile_pool(name="xs", bufs=1) as xs_pool,
                tc.tile_pool(name="kb", bufs=8) as kb_pool,
                tc.tile_pool(name="vb", bufs=8) as vb_pool,
                tc.tile_pool(name="psA", bufs=8, space="PSUM") as psA,
            ):
                w_t = {}
                for wname in ("k", "v", "q"):
                    wd = DA if wname == "v" else D
                    w_t[wname] = wpool.tile([P, EO, wd], BF, tag="w",
                                            name=f"w_{wname}")
                xq0 = xs_pool.tile([P, EO, 512], BF, tag="xs0", name="xq0")
                xq1 = xs_pool.tile([P, EO, 512], BF, tag="xs1", name="xq1")
                # startup-critical DMAs first, per-eo granularity: the
                # warmup matmuls need only the eo=0 slices of x and Wk.
                for eo in range(EO):
                    nc.gpsimd.dma_start(xq0[:, eo, :], xT_r[:, eo, 0:512])
                    if eo == 0:
                        nc.sync.dma_start(
                            w_t["k"][:, 0, 0:512], w_r["k"][:, 0, 0:512])
                        nc.sync.dma_start(
                            w_t["k"][:, 0, 512:D], w_r["k"][:, 0, 512:D])
                    else:
                        nc.sync.dma_start(
                            w_t["k"][:, eo, :], w_r["k"][:, eo, :])
                    if eo == 1:
                        nc.gpsimd.dma_start(bk_t[:], bk[:])
                nc.scalar.dma_start(xq1[:], xT_r[:, :, 512:1024])
                nc.gpsimd.dma_start(bv_t[:], bv[:])
                nc.gpsimd.dma_start(bq_t[:], bq[:])
                nc.sync.dma_start(w_t["v"][:], w_r["v"][:])
                nc.sync.dma_start(w_t["q"][:], w_r["q"][:])

                # K-own: kT[d, t_own] for own 1024-t half, to bounce, AG.
                ps0 = [psA.tile([P, 512], F32, tag="ps", name=f"ps0_{do}")
                       for do in range(DO)]
                for eo in range(EO):
                    for do in range(DO):
                        nc.tensor.matmul(
                            ps0[do][:], w_t["k"][:, eo, do * P:(do + 1) * P],
                            xq0[:, eo, :],
                            start=(eo == 0), stop=(eo == EO - 1),
                        )
                for do in range(DO):
                    kb = kb_pool.tile([P, 512], BF, tag="kb")
                    nc.scalar.activation(
                        kb[:], ps0[do][:], Ident, bias=bk_t[:, do:do + 1])
                    nc.scalar.dma_start(k_bnc[0][:, do, :], kb[:])
                nc.gpsimd.collective_compute(
                    "AllGather", mybir.AluOpType.bypass,
                    replica_groups=GROUPS,
                    ins=[k_bnc[0][:]], outs=[ag_k[0][:]],
                    unique_tensors="Yes",
                )
                for g in range(2):
                    for hh in range(2):
                        nc.sync.dma_start(
                            kT_t[:, :, g * SQ + hh * 256:
                                 g * SQ + (hh + 1) * 256],
                            ag_k[0][g][:, :, hh * 256:(hh + 1) * 256])
                for do in range(DO):
                    ps = psA.tile([P, 512], F32, tag="ps")
                    for eo in range(EO):
                        nc.tensor.matmul(
                            ps[:], w_t["k"][:, eo, do * P:(do + 1) * P],
                            xq1[:, eo, :],
                            start=(eo == 0), stop=(eo == EO - 1),
                        )
                    kb = kb_pool.tile([P, 512], BF, tag="kb")
                    nc.scalar.activation(
                        kb[:], ps[:], Ident, bias=bk_t[:, do:do + 1])
                    nc.scalar.dma_start(k_bnc[1][:, do, :], kb[:])
                nc.gpsimd.collective_compute(
                    "AllGather", mybir.AluOpType.bypass,
                    replica_groups=GROUPS,
                    ins=[k_bnc[1][:]], outs=[ag_k[1][:]],
                    unique_tensors="Yes",
                )
                for g in range(2):
                    nc.sync.dma_start(
                        kT_t[:, :, g * SQ + 512: g * SQ + 1024], ag_k[1][g])

                # V-own: v[t_own, d] (+ ones cols via bias), to bounce, AG
                # per 512-row half. Plain per-(tc,ck) chains: single-bank
                # allocation lets V start as soon as one K PSUM bank frees.
                for tc_i in range(TB * 4):
                    tb, tcl = divmod(tc_i, 4)
                    xv_t = xq0 if tc_i < 4 else xq1
                    for ck in range(3):
                        ps = psA.tile([P, DC], F32, tag="ps")
                        for eo in range(EO):
                            nc.tensor.matmul(
                                ps[:], xv_t[:, eo, tcl * P:(tcl + 1) * P],
                                w_t["v"][:, eo, ck * DC:(ck + 1) * DC],
                                start=(eo == 0), stop=(eo == EO - 1),
                            )
                        nc.vector.tensor_add(
                            ps[:], ps[:], bv_t[:, ck * DC:(ck + 1) * DC])
                        vb = vb_pool.tile([P, DC], BF, tag="vb")
                        nc.scalar.activation(vb[:], ps[:], Ident)
                        nc.scalar.dma_start(
                            v_bnc_r[tb][:, tcl, ck * DC:(ck + 1) * DC], vb[:])
                    if tc_i == 3 or tc_i == 7:
                        tb_done = tc_i // 4
                        nc.gpsimd.collective_compute(
                            "AllGather", mybir.AluOpType.bypass,
                            replica_groups=GROUPS,
                            ins=[v_bnc[tb_done][:]], outs=[ag_v[tb_done][:]],
                            unique_tensors="Yes",
                        )
                        for g in range(2):
                            nc.sync.dma_start(
                                vF_t[:, g * 8 + tb_done * 4:
                                     g * 8 + (tb_done + 1) * 4, :],
                                ag_v_r[tb_done][g],
                            )

                # Q projection: qT[d, s] = Wq @ x^T (+ bq per-partition)
                for sb in range(SB):
                    xq = xq0 if sb == 0 else xq1
                    for do in range(DO):
                        ps = psA.tile([P, 512], F32, tag="ps")
                        for eo in range(EO):
                            nc.tensor.matmul(
                                ps[:], w_t["q"][:, eo, do * P:(do + 1) * P],
                                xq[:, eo, :],
                                start=(eo == 0), stop=(eo == EO - 1),
                            )
                        nc.scalar.activation(
                            qT_t[:, do, sb * 512:(sb + 1) * 512], ps[:],
                            Ident, bias=bq_t[:, do:do + 1],
                        )

            # ---- Attention ----
            with (
                tc.tile_pool(name="eT", bufs=1) as eT_pool,
                tc.tile_pool(name="ot", bufs=3) as ot_pool,
                tc.tile_pool(name="rc", bufs=4) as rc_pool,
                tc.tile_pool(name="psS", bufs=5, space="PSUM") as psS,
                tc.tile_pool(name="psO", bufs=3, space="PSUM") as psO,
            ):
                eTs = [eT_pool.tile([P, TC, 512], BF, tag=f"eT{sb}",
                                    name=f"eT{sb}")
                       for sb in range(SB)]
                # scoresT[t, s] then eT = exp(scoresT / sqrt(D)); both query
                # blocks share each kT stationary slice back-to-back. tc
                # order follows AllGather arrival: AG-K0 carries global
                # chunks {0-3, 8-11}, AG-K1 {4-7, 12-15}.
                for tc_i in (0, 1, 2, 3, 8, 9, 10, 11,
                             4, 5, 6, 7, 12, 13, 14, 15):
                    for sb in range(SB):
                        ps = psS.tile([P, 512], F32, tag="ps")
                        for do in range(DO):
                            nc.tensor.matmul(
                                ps[:], kT_t[:, do, tc_i * P:(tc_i + 1) * P],
                                qT_t[:, do, sb * 512:(sb + 1) * 512],
                                start=(do == 0), stop=(do == DO - 1),
                            )
                        nc.scalar.activation(
                            eTs[sb][:, tc_i, :], ps[:], Exp, scale=inv_sqrt_d)

                # PV in 3 d-chunks of 344 from SBUF-resident V; chunk 2
                # carries the ones-columns => softmax denominator.
                for sb in range(SB):
                    eT = eTs[sb]
                    recips = [None] * 4
                    for ck in (2, 0, 1):
                        for ss in range(4):
                            s_lo = ss * P
                            po = psO.tile([P, DC], F32, tag="po")
                            for j, tc_i in enumerate((0, 1, 2, 3, 8, 9, 10, 11,
                                                      4, 5, 6, 7, 12, 13, 14, 15)):
                                nc.tensor.matmul(
                                    po[:], eT[:, tc_i, s_lo:s_lo + P],
                                    vF_t[:, tc_i, ck * DC:(ck + 1) * DC],
                                    start=(j == 0), stop=(j == TC - 1),
                                )
                            if ck == 2:
                                recip = rc_pool.tile([P, 1], F32, tag="recip")
                                nc.vector.reciprocal(
                                    recip[:], po[:, D - 2 * DC:D - 2 * DC + 1])
                                recips[ss] = recip
                                width = D - 2 * DC
                            else:
                                width = DC
                            o_t = ot_pool.tile([P, DC], F32, tag="ot")
                            nc.vector.tensor_scalar_mul(
                                o_t[:, :width], po[:, :width], recips[ss][:])
                            nc.gpsimd.dma_start(
                                out[ck, sb * 512 + s_lo: sb * 512 + s_lo + P,
                                    :width],
                                o_t[:, :width],
                            )

    nc.compile()
    return nc


def _get_nc():
    global _NC
    if _NC is None:
        _NC = _build()
    return _NC


def kernel(x, Wq, bq, Wk, bk, Wv, bv):
    global LAST_RESULT
    bf16 = ml_dtypes.bfloat16
    x = np.ascontiguousarray(np.asarray(x, dtype=np.float32))
    Wq = np.asarray(Wq, dtype=np.float32)
    Wk = np.asarray(Wk, dtype=np.float32)
    Wv = np.asarray(Wv, dtype=np.float32)
    bq = np.asarray(bq, dtype=np.float32)
    bk = np.asarray(bk, dtype=np.float32)
    bv = np.asarray(bv, dtype=np.float32)

    wqT = np.ascontiguousarray(Wq.T.astype(bf16))
    wkT = np.ascontiguousarray(Wk.T.astype(bf16))
    wvT = np.zeros((E, DA), dtype=bf16)
    wvT[:, :D] = Wv.T.astype(bf16)
    bq_r = np.ascontiguousarray(bq.reshape(DO, P).T)
    bk_r = np.ascontiguousarray(bk.reshape(DO, P).T)
    bv_aug = np.concatenate([bv, np.ones(DA - D, np.float32)])
    bv_r = np.ascontiguousarray(np.broadcast_to(bv_aug, (P, DA)))

    x_bf = x.astype(bf16)

    in_maps = []
    for c in range(N_CORES):
        b, h = divmod(c, 2)
        in_maps.append({
            "xT": np.ascontiguousarray(x_bf[b, h * SQ:(h + 1) * SQ, :].T),
            "wqT": wqT, "wkT": wkT, "wvT": wvT,
            "bq": bq_r, "bk": bk_r, "bv": bv_r,
        })

    nc = _get_nc()
    res = run_bass_kernel_spmd(nc, in_maps, list(range(N_CORES)), trace=TRACE)
    LAST_RESULT = res

    out = np.empty((B, S, D), dtype=np.float32)
    for c in range(N_CORES):
        b, h = divmod(c, 2)
        o = res.results[c]["out"]
        out[b, h * SQ:(h + 1) * SQ, 0:DC] = o[0]
        out[b, h * SQ:(h + 1) * SQ, DC:2 * DC] = o[1]
        out[b, h * SQ:(h + 1) * SQ, 2 * DC:D] = o[2][:, :D - 2 * DC]
    return out
